# revision 6
# baseline (speedup 1.0000x reference)
"""Trainium2 Bass kernel for nn_ClassificationModel (frame bi-RNN -> utterance bi-GRU -> FC -> pack).

Self-contained: hardcodes shapes, shards inputs across 8 NeuronCores on the host
(2 utterances/core, fully data-parallel, no collectives), runs one SPMD bass
program, and reassembles/packs the full output on the host.
"""
import os
import sys
from contextlib import ExitStack

import numpy as np

sys.path.insert(0, '/opt/trn_rl_repo')

import concourse.bass as bass          # noqa: E402
import concourse.tile as tile          # noqa: E402
import concourse.mybir as mybir        # noqa: E402
from concourse import bacc             # noqa: E402
from concourse.bass_utils import run_bass_kernel_spmd  # noqa: E402

F32 = mybir.dt.float32
AF = mybir.ActivationFunctionType
ALU = mybir.AluOpType

B, F, T, M, H, C = 16, 512, 32, 128, 128, 61
NCORES, U = 8, 2
N = U * F                 # 1024 frame-columns per core, col = f*U + u
NCHUNK = 4
CC = N // NCHUNK          # 256 cols per frame chunk

_cache = {}


def _build_program():
    nc = bacc.Bacc("TRN2", target_bir_lowering=False, debug=False)

    def din(name, shape):
        return nc.dram_tensor(name, shape, F32, kind="ExternalInput").ap()

    xT = din("xT", [NCHUNK, T, M, CC])
    w0ih = din("w0ih", [2, 128, 128])
    w0hh = din("w0hh", [2, 128, 128])
    b0 = din("b0", [2, 128, 1])
    w1ih = din("w1ih", [2, 2, 128, 128])
    w1hh = din("w1hh", [2, 128, 128])
    b1 = din("b1", [2, 128, 1])
    gwih = [din(f"gwih{l}", [2, 3, 2, 128, 128]) for l in range(2)]
    gwhh = [din(f"gwhh{l}", [2, 3, 128, 128]) for l in range(2)]
    gbi = [din(f"gbi{l}", [2, 3, 128, 1]) for l in range(2)]
    gbhn = [din(f"gbhn{l}", [2, 128, 1]) for l in range(2)]
    fcw = din("fcw", [2, 128, 61])
    fcb = din("fcb", [61, 1])
    logits = nc.dram_tensor("logits", [C, N], F32, kind="ExternalOutput").ap()

    with tile.TileContext(nc) as tc, ExitStack() as ctx:
        cpool = ctx.enter_context(tc.tile_pool(name="consts", bufs=1))

        def ctile(src_ap, shape, tag):
            t = cpool.tile(shape, F32, tag=tag, name=tag)
            nc.sync.dma_start(t[:], src_ap)
            return t

        w0ih_t = [ctile(w0ih[d], [128, 128], f"w0ih{d}") for d in range(2)]
        w0hh_t = [ctile(w0hh[d], [128, 128], f"w0hh{d}") for d in range(2)]
        b0_t = [ctile(b0[d], [128, 1], f"b0{d}") for d in range(2)]
        w1ih_t = [[ctile(w1ih[d, k], [128, 128], f"w1ih{d}{k}") for k in range(2)]
                  for d in range(2)]
        w1hh_t = [ctile(w1hh[d], [128, 128], f"w1hh{d}") for d in range(2)]
        b1_t = [ctile(b1[d], [128, 1], f"b1{d}") for d in range(2)]
        gwih_t = [[[[ctile(gwih[l][d, g, k], [128, 128], f"gwih{l}{d}{g}{k}")
                     for k in range(2)] for g in range(3)] for d in range(2)]
                  for l in range(2)]
        gwhh_t = [[[ctile(gwhh[l][d, g], [128, 128], f"gwhh{l}{d}{g}")
                    for g in range(3)] for d in range(2)] for l in range(2)]
        gbi_t = [[[ctile(gbi[l][d, g], [128, 1], f"gbi{l}{d}{g}")
                   for g in range(3)] for d in range(2)] for l in range(2)]
        gbhn_t = [[ctile(gbhn[l][d], [128, 1], f"gbhn{l}{d}") for d in range(2)]
                  for l in range(2)]
        fcw_t = [ctile(fcw[k], [128, 61], f"fcw{k}") for k in range(2)]
        fcb_t = ctile(fcb, [61, 1], "fcb")

        zeros2 = cpool.tile([128, 2], F32, tag="zeros2", name="zeros2")
        nc.vector.memset(zeros2[:], 0.0)

        persist = ctx.enter_context(tc.tile_pool(name="persist", bufs=1))
        frames_f = persist.tile([128, N], F32, tag="frames_f", name="frames_f")
        frames_b = persist.tile([128, N], F32, tag="frames_b", name="frames_b")

        # ---------------- Phase A: frame-level bi-RNN ----------------
        with ExitStack() as phase_a:
            xpool = phase_a.enter_context(tc.tile_pool(name="xchunk", bufs=1))
            o0pool = phase_a.enter_context(tc.tile_pool(name="o0", bufs=1))
            h1pool = phase_a.enter_context(tc.tile_pool(name="h1", bufs=3))
            fpsum = phase_a.enter_context(
                tc.tile_pool(name="fpsum", bufs=2, space="PSUM"))

            def tsl(t):
                return slice(t * CC, (t + 1) * CC)

            for ch in range(NCHUNK):
                xt = xpool.tile([128, T * CC], F32, tag="x", name="x")
                for t in range(T):
                    nc.sync.dma_start(xt[:, tsl(t)], xT[ch, t])
                o0f = o0pool.tile([128, T * CC], F32, tag="o0f", name="o0f")
                o0b = o0pool.tile([128, T * CC], F32, tag="o0b", name="o0b")
                # L0 forward
                for t in range(T):
                    ps = fpsum.tile([128, CC], F32, tag="psA", name="psA")
                    nc.tensor.matmul(ps[:], w0ih_t[0][:], xt[:, tsl(t)],
                                     start=True, stop=(t == 0))
                    if t > 0:
                        nc.tensor.matmul(ps[:], w0hh_t[0][:], o0f[:, tsl(t - 1)],
                                         start=False, stop=True)
                    nc.scalar.activation(o0f[:, tsl(t)], ps[:], AF.Tanh,
                                         bias=b0_t[0][:])
                # L0 backward
                for t in reversed(range(T)):
                    ps = fpsum.tile([128, CC], F32, tag="psB", name="psB")
                    nc.tensor.matmul(ps[:], w0ih_t[1][:], xt[:, tsl(t)],
                                     start=True, stop=(t == T - 1))
                    if t < T - 1:
                        nc.tensor.matmul(ps[:], w0hh_t[1][:], o0b[:, tsl(t + 1)],
                                         start=False, stop=True)
                    nc.scalar.activation(o0b[:, tsl(t)], ps[:], AF.Tanh,
                                         bias=b0_t[1][:])
                # L1 forward (only final h needed)
                hprev = None
                for t in range(T):
                    ps = fpsum.tile([128, CC], F32, tag="psC", name="psC")
                    nc.tensor.matmul(ps[:], w1ih_t[0][0][:], o0f[:, tsl(t)],
                                     start=True, stop=False)
                    nc.tensor.matmul(ps[:], w1ih_t[0][1][:], o0b[:, tsl(t)],
                                     start=False, stop=(t == 0))
                    if t > 0:
                        nc.tensor.matmul(ps[:], w1hh_t[0][:], hprev[:],
                                         start=False, stop=True)
                    if t == T - 1:
                        nc.scalar.activation(frames_f[:, ch * CC:(ch + 1) * CC],
                                             ps[:], AF.Tanh, bias=b1_t[0][:])
                    else:
                        h1 = h1pool.tile([128, CC], F32, tag="h1", name="h1")
                        nc.scalar.activation(h1[:], ps[:], AF.Tanh, bias=b1_t[0][:])
                        hprev = h1
                # L1 backward: output at last frame needs a single step
                ps = fpsum.tile([128, CC], F32, tag="psD", name="psD")
                nc.tensor.matmul(ps[:], w1ih_t[1][0][:], o0f[:, tsl(T - 1)],
                                 start=True, stop=False)
                nc.tensor.matmul(ps[:], w1ih_t[1][1][:], o0b[:, tsl(T - 1)],
                                 start=False, stop=True)
                nc.scalar.activation(frames_b[:, ch * CC:(ch + 1) * CC], ps[:],
                                     AF.Tanh, bias=b1_t[1][:])

        # ---------------- Phase B: utterance bi-GRU (2 layers) ----------------
        gout_t = [[persist.tile([128, N], F32, tag=f"gout{l}{d}", name=f"gout{l}{d}") for d in range(2)]
                  for l in range(2)]
        with ExitStack() as phase_b:
            gipool = phase_b.enter_context(tc.tile_pool(name="gi", bufs=1))
            gps = phase_b.enter_context(
                tc.tile_pool(name="gps", bufs=2, space="PSUM"))
            sp = phase_b.enter_context(tc.tile_pool(name="gsmall", bufs=4))
            spsum = phase_b.enter_context(
                tc.tile_pool(name="spsum", bufs=3, space="PSUM"))

            inf, inb = frames_f, frames_b
            for l in range(2):
                girz = [gipool.tile([128, 4 * F], F32, tag=f"girz{d}", name=f"girz{d}")
                        for d in range(2)]
                gin = [gipool.tile([128, 2 * F], F32, tag=f"gin{d}", name=f"gin{d}")
                       for d in range(2)]
                # bulk input-part precompute: gi = Wih @ frames + bias
                for d in range(2):
                    for g in range(3):
                        for hc in range(2):
                            sl = slice(hc * 512, (hc + 1) * 512)
                            ps = gps.tile([128, 512], F32, tag="gips", name="gips")
                            nc.tensor.matmul(ps[:], gwih_t[l][d][g][0][:],
                                             inf[:, sl], start=True, stop=False)
                            nc.tensor.matmul(ps[:], gwih_t[l][d][g][1][:],
                                             inb[:, sl], start=False, stop=True)
                            if g < 2:
                                out_ap = girz[d][:].rearrange(
                                    "p (f x) -> p f x", x=4)[
                                    :, hc * 256:(hc + 1) * 256, 2 * g:2 * g + 2]
                                in_ap = ps[:].rearrange("p (f x) -> p f x", x=2)
                                nc.scalar.activation(out_ap, in_ap, AF.Identity,
                                                     bias=gbi_t[l][d][g][:])
                            else:
                                nc.scalar.activation(gin[d][:, sl], ps[:],
                                                     AF.Identity,
                                                     bias=gbi_t[l][d][g][:])
                # recurrence
                for k in range(F):
                    for d in range(2):
                        f = k if d == 0 else F - 1 - k
                        if k == 0:
                            h_prev = zeros2[:]
                        else:
                            fp = f - 1 if d == 0 else f + 1
                            h_prev = gout_t[l][d][:, 2 * fp:2 * fp + 2]
                        ps = spsum.tile([128, 6], F32, tag=f"ps{d}", name=f"ps{d}")
                        for g in range(3):
                            nc.tensor.matmul(ps[:, 2 * g:2 * g + 2],
                                             gwhh_t[l][d][g][:], h_prev,
                                             start=True, stop=True)
                        s = sp.tile([128, 4], F32, tag=f"s{d}", name=f"s{d}")
                        nc.vector.tensor_add(s[:], ps[:, 0:4],
                                             girz[d][:, 4 * f:4 * f + 4])
                        rz = sp.tile([128, 4], F32, tag=f"rz{d}", name=f"rz{d}")
                        nc.scalar.activation(rz[:], s[:], AF.Sigmoid)
                        t1 = sp.tile([128, 2], F32, tag=f"t1{d}", name=f"t1{d}")
                        nc.vector.scalar_tensor_tensor(
                            t1[:], ps[:, 4:6], gbhn_t[l][d][:], rz[:, 0:2],
                            ALU.add, ALU.mult)
                        t2 = sp.tile([128, 2], F32, tag=f"t2{d}", name=f"t2{d}")
                        nc.vector.tensor_add(t2[:], t1[:],
                                             gin[d][:, 2 * f:2 * f + 2])
                        n_ = sp.tile([128, 2], F32, tag=f"n{d}", name=f"n{d}")
                        nc.scalar.activation(n_[:], t2[:], AF.Tanh)
                        u = sp.tile([128, 2], F32, tag=f"u{d}", name=f"u{d}")
                        nc.vector.tensor_sub(u[:], h_prev, n_[:])
                        v = sp.tile([128, 2], F32, tag=f"v{d}", name=f"v{d}")
                        nc.vector.tensor_mul(v[:], u[:], rz[:, 2:4])
                        nc.vector.tensor_add(gout_t[l][d][:, 2 * f:2 * f + 2],
                                             v[:], n_[:])
                inf, inb = gout_t[l][0], gout_t[l][1]

        # ---------------- Phase C: FC + output ----------------
        with ExitStack() as phase_c:
            fps = phase_c.enter_context(
                tc.tile_pool(name="fcpsum", bufs=2, space="PSUM"))
            lpool = phase_c.enter_context(tc.tile_pool(name="lsb", bufs=1))
            lsb = lpool.tile([C, N], F32, tag="lsb", name="lsb")
            for hc in range(2):
                sl = slice(hc * 512, (hc + 1) * 512)
                ps = fps.tile([C, 512], F32, tag="fcps", name="fcps")
                nc.tensor.matmul(ps[:], fcw_t[0][:], gout_t[1][0][:, sl],
                                 start=True, stop=False)
                nc.tensor.matmul(ps[:], fcw_t[1][:], gout_t[1][1][:, sl],
                                 start=False, stop=True)
                nc.scalar.activation(lsb[:, sl], ps[:], AF.Identity,
                                     bias=fcb_t[:])
            nc.sync.dma_start(logits, lsb[:])

    nc.compile()
    return nc


def _prep_common(inp):
    f32 = np.float32
    c = {}
    c["w0ih"] = np.ascontiguousarray(
        np.stack([inp["rnn1_l0_Wih"][d].T for d in range(2)]), dtype=f32)
    c["w0hh"] = np.ascontiguousarray(
        np.stack([inp["rnn1_l0_Whh"][d].T for d in range(2)]), dtype=f32)
    c["b0"] = np.ascontiguousarray(
        (inp["rnn1_l0_bih"] + inp["rnn1_l0_bhh"])[:, :, None], dtype=f32)
    w1 = np.stack([inp["rnn1_l1_Wih"][d].T for d in range(2)])
    c["w1ih"] = np.ascontiguousarray(w1.reshape(2, 2, 128, 128), dtype=f32)
    c["w1hh"] = np.ascontiguousarray(
        np.stack([inp["rnn1_l1_Whh"][d].T for d in range(2)]), dtype=f32)
    c["b1"] = np.ascontiguousarray(
        (inp["rnn1_l1_bih"] + inp["rnn1_l1_bhh"])[:, :, None], dtype=f32)
    for l in range(2):
        wih = inp[f"gru_l{l}_Wih"]
        whh = inp[f"gru_l{l}_Whh"]
        bih = inp[f"gru_l{l}_bih"]
        bhh = inp[f"gru_l{l}_bhh"]
        gwih_a = np.zeros((2, 3, 2, 128, 128), f32)
        gwhh_a = np.zeros((2, 3, 128, 128), f32)
        gbi_a = np.zeros((2, 3, 128, 1), f32)
        gbhn_a = np.zeros((2, 128, 1), f32)
        for d in range(2):
            for g in range(3):
                wt = wih[d, g * 128:(g + 1) * 128, :].T
                gwih_a[d, g] = wt.reshape(2, 128, 128)
                gwhh_a[d, g] = whh[d, g * 128:(g + 1) * 128, :].T
                if g < 2:
                    gbi_a[d, g, :, 0] = (bih[d, g * 128:(g + 1) * 128]
                                         + bhh[d, g * 128:(g + 1) * 128])
                else:
                    gbi_a[d, g, :, 0] = bih[d, g * 128:(g + 1) * 128]
            gbhn_a[d, :, 0] = bhh[d, 2 * 128:3 * 128]
        c[f"gwih{l}"] = gwih_a
        c[f"gwhh{l}"] = gwhh_a
        c[f"gbi{l}"] = gbi_a
        c[f"gbhn{l}"] = gbhn_a
    c["fcw"] = np.ascontiguousarray(
        np.asarray(inp["fc_W"], dtype=f32).T.reshape(2, 128, 61))
    c["fcb"] = np.ascontiguousarray(
        np.asarray(inp["fc_b"], dtype=f32)[:, None])
    return c


def _shard_x(x):
    xs = np.asarray(x, dtype=np.float32).reshape(B, F, T, M)
    shards = []
    for cidx in range(NCORES):
        xc = xs[U * cidx:U * cidx + U]               # [U, F, T, M]
        xt = xc.transpose(2, 3, 1, 0)                # [T, M, F, U]
        xt = xt.reshape(T, M, NCHUNK, F // NCHUNK, U)
        xt = xt.transpose(2, 0, 1, 3, 4).reshape(NCHUNK, T, M, CC)
        shards.append(np.ascontiguousarray(xt))
    return shards


def _install_ntff_hook_shim():
    """Provide antenv.axon_hooks (missing in this image) so trace=True can
    capture NTFF profiles through the axon PJRT .so."""
    import types
    import ctypes
    import contextlib
    if "antenv.axon_hooks" in sys.modules:
        return
    so_path = "/opt/axon/libaxon_pjrt.so"
    if not os.path.exists(so_path):
        return
    lib = ctypes.CDLL(so_path)
    if not hasattr(lib, "axon_start_nrt_profile"):
        return
    lib.axon_start_nrt_profile.argtypes = [
        ctypes.POINTER(ctypes.c_int64), ctypes.c_size_t]
    lib.axon_start_nrt_profile.restype = ctypes.c_int64
    lib.axon_stop_nrt_profile.argtypes = [ctypes.c_char_p]
    lib.axon_stop_nrt_profile.restype = ctypes.c_int64

    @contextlib.contextmanager
    def _hook(output_dir, device_ids):
        import jax
        jax.devices()
        if device_ids:
            ids = (ctypes.c_int64 * len(device_ids))(*device_ids)
            rc = lib.axon_start_nrt_profile(ids, len(device_ids))
        else:
            rc = lib.axon_start_nrt_profile(None, 0)
        if rc != 0:
            raise RuntimeError(f"axon_start_nrt_profile rc={rc}")
        try:
            yield
        finally:
            n = lib.axon_stop_nrt_profile(str(output_dir).encode())
            print(f"ntff profile: {n} file(s) -> {output_dir}")

    mod = types.ModuleType("antenv.axon_hooks")
    mod.get_axon_ntff_profile_hook = lambda: _hook
    mod.set_axon_ntff_profile_hook = lambda h: None
    sys.modules["antenv.axon_hooks"] = mod


def kernel(**inputs):
    inputs = {k: np.asarray(v) for k, v in inputs.items()}
    if "nc" not in _cache:
        _cache["nc"] = _build_program()
    nc = _cache["nc"]

    common = _prep_common(inputs)
    rename = {f"gwih{l}": f"gwih{l}" for l in range(2)}
    del rename
    shards = _shard_x(inputs["x"])
    in_maps = []
    for cidx in range(NCORES):
        m = {"xT": shards[cidx]}
        for k, v in common.items():
            m[k] = v
        in_maps.append(m)

    trace = os.environ.get("KERNEL_TRACE", "0") == "1"
    if trace:
        _install_ntff_hook_shim()
    res = run_bass_kernel_spmd(nc, in_maps, list(range(NCORES)), trace=trace)
    _cache["last_results"] = res

    logits_all = np.empty((B, F, C), np.float32)
    for cidx in range(NCORES):
        lg = res.results[cidx]["logits"].reshape(C, F, U)
        for u in range(U):
            logits_all[U * cidx + u] = lg[:, :, u].T
    Ls = np.asarray(inputs["lengths"]).astype(np.int64)
    return np.concatenate([logits_all[i, :Ls[i]] for i in range(B)], axis=0)
